# revision 18
# baseline (speedup 1.0000x reference)
"""Trainium2 Bass kernel for nn_AttentionSiphon.

Reference computes: tokens = x @ W_map + b_map; concat [time, cluster, tokens];
LayerNorm; per-head q/k projections; softmax(q k^T / sqrt(dh)); mean over heads;
returns rows 0 and 1 of the [B,S,S] head-mean attention.

Only attention rows 0/1 are returned, and their queries come from the
(batch-independent) time/cluster tokens. So the per-head attention collapses:

  score[j, c=2h+r] = LN(token_j) . (Wk[h] @ q_r[h])   (+ constants)

which is one [D, 34] matmul against the token matrix (columns 32/33 gather the
LN statistics' linear parts). The heavy device work is the token projection
x @ W_map ([8184,512]@[512,1024]) plus the sum of squared tokens for the LN
variance, on 8 NeuronCores with rows sharded 1024/core (tail zero-padded).

Device output per core: [35, 1024] = 32 score columns + col-sum + b_map-cross
+ sumsq, per row. The tiny softmax epilogue ([4,16,2,2048]) runs on host.
"""

import os
import sys

sys.path.insert(0, "/opt/trn_rl_repo")

import numpy as np
import ml_dtypes

B, N, IN_D = 4, 2046, 512
D, H, DH = 1024, 16, 64
S = N + 2
EPS = 1e-5
NCORES = 8
JPC = 1024            # padded rows per core
JTOT = NCORES * JPC   # 8192 (8184 real rows + 8 pad)
NAUG = 34             # 32 score cols + colsum + b_map cross
NC_OUT = 35           # + sumsq row

# Precision scheme: "bf16" (1-pass matmuls), "split" (hi/lo bf16 3-pass)
PRECISION = os.environ.get("AS_PRECISION", "split")

_PROG_CACHE = {}
LAST_RESULT = None  # BassKernelResults of the most recent run (for test harness)


def _bf16(a):
    return np.asarray(a, np.float32).astype(ml_dtypes.bfloat16)


def _split_hi_lo(a):
    a = np.asarray(a, np.float32)
    hi = a.astype(ml_dtypes.bfloat16)
    lo = (a - hi.astype(np.float32)).astype(ml_dtypes.bfloat16)
    return hi, lo


def _build_program(precision):
    import concourse.bacc as bacc
    import concourse.mybir as mybir
    from concourse import tile

    f32 = mybir.dt.float32
    bf = mybir.dt.bfloat16
    AF = mybir.ActivationFunctionType

    nc = bacc.Bacc("TRN2")

    split = precision == "split"
    map_passes = [("hi", "hi"), ("hi", "lo"), ("lo", "hi")] if split else [("", "")]
    sfx = ("hi", "lo") if split else ("",)

    xt = {s: nc.dram_tensor(f"xt{s}", [128, 2, 4, 512], bf, kind="ExternalInput")
          for s in sfx}
    wm = {s: nc.dram_tensor(f"wm{s}", [128, 8, 4, 128], bf, kind="ExternalInput")
          for s in sfx}
    va = {s: nc.dram_tensor(f"va{s}", [128, 8, NAUG], bf, kind="ExternalInput")
          for s in sfx}
    out_h = nc.dram_tensor("out", [NC_OUT, JPC], f32, kind="ExternalOutput")

    NJT = JPC // 512  # 2 j-tiles

    with tile.TileContext(nc) as tc:
        with (
            tc.tile_pool(name="cst", bufs=1) as cst,
            tc.tile_pool(name="big", bufs=1) as big,
            tc.tile_pool(name="ps_map", bufs=4, space="PSUM") as ps_map,
            tc.tile_pool(name="ps_sml", bufs=2, space="PSUM") as ps_sml,
        ):
            xt_sb = {s: big.tile([128, 2, 4, 512], bf, name=f"xt{s}_sb", tag=f"xt{s}") for s in sfx}
            wm_sb = {s: big.tile([128, 8, 4, 128], bf, name=f"wm{s}_sb", tag=f"wm{s}") for s in sfx}
            va_sb = {s: cst.tile([128, 8, NAUG], bf, name=f"va{s}_sb", tag=f"va{s}") for s in sfx}
            tb_sb = {s: big.tile([128, 8, JPC], bf, name=f"tb{s}_sb", tag=f"tb{s}") for s in sfx}
            sq_sb = {s: big.tile([128, 8, JPC], bf, name=f"sq{s}_sb", tag=f"sq{s}") for s in sfx}
            mo_sb = cst.tile([128, 1], bf)       # ones column
            out_sb = cst.tile([NAUG, JPC], f32)
            sqr_sb = cst.tile([1, JPC], f32)     # sumsq row (32-align rule)

            # DMA order matches first-use order: wm[dc0], xt[jt0], rest.
            for s in sfx:
                nc.sync.dma_start(wm_sb[s][:, 0], wm[s][:, 0])
            for s in sfx:
                nc.sync.dma_start(xt_sb[s][:, 0], xt[s][:, 0])
            for dc in range(1, 8):
                for s in sfx:
                    nc.sync.dma_start(wm_sb[s][:, dc], wm[s][:, dc])
            for s in sfx:
                nc.sync.dma_start(xt_sb[s][:, 1], xt[s][:, 1])
                nc.sync.dma_start(va_sb[s][:], va[s][:])
            nc.vector.memset(mo_sb[:], 1.0)

            # ---- token projection: T^T[d, j] = sum_i W[i,d] * xT[i,j] ----
            for jt in range(NJT):
                for dc in range(8):
                    ps = ps_map.tile([128, 512], f32, name="psmap", tag="psmap")
                    nmm = len(map_passes) * 4
                    k = 0
                    for (ls, rs) in map_passes:
                        for i in range(4):
                            nc.tensor.matmul(
                                ps[:],
                                wm_sb[ls][:, dc, i, :],
                                xt_sb[rs][:, jt, i, :],
                                start=(k == 0),
                                stop=(k == nmm - 1),
                            )
                            k += 1
                    jsl = slice(jt * 512, (jt + 1) * 512)
                    if split:
                        nc.vector.tensor_copy(tb_sb["hi"][:, dc, jsl], ps[:])
                        nc.vector.tensor_sub(
                            tb_sb["lo"][:, dc, jsl], ps[:], tb_sb["hi"][:, dc, jsl]
                        )
                        sqf = big.tile([128, 512], f32, name="sqf",
                                       tag="sqf", bufs=16)
                        nc.scalar.activation(sqf[:], ps[:], AF.Square)
                        nc.scalar.activation(
                            sq_sb["hi"][:, dc, jsl], sqf[:], AF.Copy
                        )
                        nc.vector.tensor_sub(
                            sq_sb["lo"][:, dc, jsl], sqf[:],
                            sq_sb["hi"][:, dc, jsl],
                        )
                    else:
                        nc.vector.tensor_copy(tb_sb[""][:, dc, jsl], ps[:])
                        nc.scalar.activation(
                            sq_sb[""][:, dc, jsl], ps[:], AF.Square
                        )

            # ---- scores/stats via Vaug^T @ T^T;  sumsq via ones^T @ SQ ----
            if split:
                sc_passes = [("hi", "hi"), ("hi", "lo"), ("lo", "hi")]
                sq_passes = [("", "hi"), ("", "lo")]
            else:
                sc_passes = [("", "")]
                sq_passes = [("", "")]
            for jt in range(NJT):
                jsl = slice(jt * 512, (jt + 1) * 512)
                psy = ps_sml.tile([NAUG, 512], f32, name="psy", tag="psy")
                nmm = len(sc_passes) * 8
                k = 0
                for (ls, rs) in sc_passes:
                    for dc in range(8):
                        nc.tensor.matmul(
                            psy[:],
                            va_sb[ls][:, dc, :],
                            tb_sb[rs][:, dc, jsl],
                            start=(k == 0),
                            stop=(k == nmm - 1),
                        )
                        k += 1
                nc.vector.tensor_copy(out_sb[0:NAUG, jsl], psy[:])

                pss = ps_sml.tile([1, 512], f32, name="pss", tag="pss")
                nmm = len(sq_passes) * 8
                k = 0
                for (_, rs) in sq_passes:
                    for dc in range(8):
                        nc.tensor.matmul(
                            pss[:],
                            mo_sb[:],
                            sq_sb[rs][:, dc, jsl],
                            start=(k == 0),
                            stop=(k == nmm - 1),
                        )
                        k += 1
                nc.vector.tensor_copy(sqr_sb[:, jsl], pss[:])

            nc.sync.dma_start(out_h[0:NAUG], out_sb[:])
            nc.sync.dma_start(out_h[NAUG:NC_OUT], sqr_sb[:])

    nc.compile()
    return nc


def _host_precompute(inputs):
    x = np.asarray(inputs["x"], np.float32)
    W = np.asarray(inputs["W_map"], np.float32)
    b_map = np.asarray(inputs["b_map"], np.float32)
    g = np.asarray(inputs["ln_g"], np.float32)
    lb = np.asarray(inputs["ln_b"], np.float32)
    Wq = np.asarray(inputs["Wq"], np.float32)
    bq = np.asarray(inputs["bq"], np.float32)
    Wk = np.asarray(inputs["Wk"], np.float32)
    bk = np.asarray(inputs["bk"], np.float32)
    tt = np.asarray(inputs["time_token"], np.float32)
    ct = np.asarray(inputs["cluster_token"], np.float32)

    spec = np.concatenate([tt, ct], 0)                      # [2, D]
    mu = spec.mean(-1, keepdims=True)
    var = ((spec - mu) ** 2).mean(-1, keepdims=True)
    hspec = ((spec - mu) / np.sqrt(var + EPS) * g + lb).reshape(2, H, DH)
    q = np.einsum("rhd,hde->rhe", hspec, Wq) + bq[None]
    qs = (q / np.sqrt(DH)).astype(np.float32)               # [2,H,DH]
    kspec = np.einsum("rhd,hde->rhe", hspec, Wk) + bk[None]
    s_spec = np.einsum("rhe,the->hrt", qs, kspec)           # [H,2,2]

    v = np.einsum("hde,rhe->hdr", Wk, qs)                   # [H,DH,2]
    V = np.zeros((D, 2 * H), np.float32)
    for h in range(H):
        V[64 * h:64 * h + 64, 2 * h] = v[h, :, 0]
        V[64 * h:64 * h + 64, 2 * h + 1] = v[h, :, 1]
    c0 = np.empty(2 * H, np.float32)
    for h in range(H):
        c0[2 * h] = qs[0, h] @ bk[h]
        c0[2 * h + 1] = qs[1, h] @ bk[h]

    Vg = g[:, None] * V
    # augmented score matrix: [Vg | ones | b_map]
    Vaug = np.concatenate(
        [Vg, np.ones((D, 1), np.float32), b_map[:, None]], 1)  # [D, 34]
    consts = dict(
        pg=Vg.sum(0),
        qb=(lb[:, None] * V).sum(0),
        bVg=(b_map[:, None] * Vg).sum(0),
        bmean=b_map.mean(),
        bsq=(b_map ** 2).sum(),
        s_spec=s_spec,
        c0=c0,
    )
    return x, Vaug, W, consts


def kernel(**inputs):
    from concourse.bass_utils import run_bass_kernel_spmd

    x, Vaug, W, consts = _host_precompute(inputs)

    key = PRECISION
    if key not in _PROG_CACHE:
        _PROG_CACHE[key] = _build_program(key)
    nc = _PROG_CACHE[key]

    split = PRECISION == "split"

    xf = x.reshape(B * N, IN_D)
    xpad = np.zeros((JTOT, IN_D), np.float32)
    xpad[:B * N] = xf

    def pmaj(a, k):
        # [k*128, n] -> partition-major [128, k, n]
        return np.ascontiguousarray(
            np.asarray(a).reshape(k, 128, -1).transpose(1, 0, 2))

    def pack_wm(a):
        # [512, 1024] -> [128p, 8dc, 4i, 128]
        return np.ascontiguousarray(
            np.asarray(a).reshape(4, 128, 8, 128).transpose(1, 2, 0, 3))

    def pack_xt(a):
        # [512, 1024] -> [128p, 2jt, 4i, 512]
        return np.ascontiguousarray(
            np.asarray(a).reshape(4, 128, 2, 512).transpose(1, 2, 0, 3))

    shared = {}
    if split:
        Whi, Wlo = _split_hi_lo(W)
        Vhi, Vlo = _split_hi_lo(Vaug)
        shared["wmhi"] = pack_wm(Whi)
        shared["wmlo"] = pack_wm(Wlo)
        shared["vahi"] = pmaj(Vhi, 8)
        shared["valo"] = pmaj(Vlo, 8)
    else:
        shared["wm"] = pack_wm(_bf16(W))
        shared["va"] = pmaj(_bf16(Vaug), 8)

    in_maps = []
    for c in range(NCORES):
        xT = np.ascontiguousarray(xpad[c * JPC:(c + 1) * JPC].T)  # [512, 1024]
        m = dict(shared)
        if split:
            xh, xl = _split_hi_lo(xT)
            m["xthi"] = pack_xt(xh)
            m["xtlo"] = pack_xt(xl)
        else:
            m["xt"] = pack_xt(_bf16(xT))
        in_maps.append(m)

    trace = bool(int(os.environ.get("AS_TRACE", "0")))
    res = run_bass_kernel_spmd(nc, in_maps, list(range(NCORES)), trace=trace)
    global LAST_RESULT
    LAST_RESULT = res
    outs = [np.asarray(r["out"], np.float32) for r in res.results]

    return _epilogue(outs, consts)


def _epilogue(outs, consts):
    full = np.concatenate([o.T for o in outs], 0)[:B * N]   # [8184, 35]
    Y = full[:, 0:32]
    colsum = full[:, 32]
    bcross = full[:, 33]
    SQ = full[:, 34]

    mu = colsum / np.float32(D) + consts["bmean"]
    E2 = (SQ + 2.0 * bcross + consts["bsq"]) / np.float32(D)
    var = E2 - mu ** 2
    rstd = (1.0 / np.sqrt(var + EPS)).astype(np.float32)
    G = Y + consts["bVg"][None]
    sc = (rstd[:, None] * G
          - (rstd * mu)[:, None] * consts["pg"][None]
          + consts["qb"][None] + consts["c0"][None])
    sc = sc.reshape(B, N, H, 2).transpose(0, 2, 3, 1)       # [B,H,2,N]

    scores = np.empty((B, H, 2, S), np.float32)
    scores[:, :, :, 2:] = sc
    scores[:, :, :, 0:2] = consts["s_spec"][None]

    m = scores - scores.max(-1, keepdims=True)
    e = np.exp(m)
    attn = e / e.sum(-1, keepdims=True)
    mm = attn.mean(1)                                       # [B,2,S]
    return (np.ascontiguousarray(mm[:, 0, :]),
            np.ascontiguousarray(mm[:, 1, :]))
